# revision 2
# baseline (speedup 1.0000x reference)
"""Trainium2 Bass kernel for nn_DeepSymmetricGCN1dBlock — v2.

3-layer GCN block over a shared 2048-node graph, 32 graph copies (b=4, n=8),
channels 128->256->256->256, per-element branch + symmetric max-pooled branch,
training-mode BatchNorm, ReLU.

v2 strategy (vs baseline):
- A = D~ @ Acnt @ D~ factorization: the shipped matrix Acnt (= edge counts
  + 2I) has exact small-integer entries, stored bf16 (8 MB, SBUF-resident all
  3 layers).  src-side dis rides the node-major drains as a per-partition ACT
  scale; dst-side dis is a DVE multiply with a broadcast disrow tile applied
  to the (f32) aggregation output BEFORE the weight matmul / BN stats.
- All matmul moving operands bf16 or f32r (both 1 cyc/row); stationary
  operands amortized: loop st-outer streaming 1024 cols per load.
- y1 stored bf16; agg kept f32 (tf32 rhs) for accuracy.
- BN gamma sign folded into the y1 drain scale (s = sign(g)), making the
  pooled-branch max-commute trick (max relu(a1 y1 + y2) = relu(a1 max y1 +
  y2)) valid for any gamma sign.
- Per layer, the tiny BN-stats AllReduce(add, 8 cores) is issued FIRST, then
  the pooled pair AllReduce(max) of localmax(s*y1) (bf16, 1 MB) overlaps with
  the element-branch P3 work.  The initial pooled-input pair-max overlaps the
  first layer's element matmuls.
"""

import sys

if "/opt/trn_rl_repo" not in sys.path:
    sys.path.insert(0, "/opt/trn_rl_repo")

import numpy as np

import concourse.bass as bass
import concourse.bacc as bacc
import concourse.mybir as mybir
import concourse.tile as tile
from concourse.bass_utils import run_bass_kernel_spmd

f32 = mybir.dt.float32
f32r = mybir.dt.float32r
bf16 = mybir.dt.bfloat16
AF = mybir.ActivationFunctionType
OP = mybir.AluOpType

B, N, L, E = 4, 8, 2048, 16384
CH = [128, 256, 256, 256]
EPS = 1e-5
NCORES = 8
GPC = 4            # graph copies per core
LT = 16            # node tiles (2048/128)
CNT_E = 32 * L     # element-branch BN count
CNT_P = 8 * L      # pooled-branch BN count (double-counted by pairs)
AGGC = 1024        # agg psum chunk columns

PAIRS = [[0, 1], [2, 3], [4, 5], [6, 7]]
ALL8 = [list(range(NCORES))]

import os
_PROFILE = False
_SIMULATE = False
_NO_CC = os.environ.get("K_NO_CC", "0") == "1"
_CACHE = {}


def _cc(nc, kind, op, groups, bi, bo):
    if _NO_CC:
        nc.sync.dma_start(bo[:], bi[:])
    else:
        nc.gpsimd.collective_compute(kind, op, replica_groups=groups,
                                     ins=[bi[:].opt()], outs=[bo[:].opt()])


def _emit(tc, nc, io):
    sync, vec, act, gp, te = nc.sync, nc.vector, nc.scalar, nc.gpsimd, nc.tensor
    from contextlib import ExitStack

    ctx = ExitStack()
    with ctx:
        sb = ctx.enter_context(tc.tile_pool(name="sb", bufs=1))
        sb_slot = ctx.enter_context(tc.tile_pool(name="slots", bufs=GPC))
        sb_y1 = ctx.enter_context(tc.tile_pool(name="y1", bufs=GPC + 1))
        sb_small = ctx.enter_context(tc.tile_pool(name="small", bufs=24))
        sb_w = ctx.enter_context(tc.tile_pool(name="w", bufs=6))
        ps_a = ctx.enter_context(tc.tile_pool(name="psa", bufs=2, space="PSUM"))
        ps_y = ctx.enter_context(tc.tile_pool(name="psy", bufs=2, space="PSUM"))
        ps_t = ctx.enter_context(tc.tile_pool(name="pst", bufs=2, space="PSUM"))
        dram = ctx.enter_context(tc.tile_pool(name="dram", bufs=1, space="DRAM"))

        # ---- persistent SBUF tiles -------------------------------------
        Ab = sb.tile([128, LT * L], bf16, tag="Ab")            # 8 MB resident
        slots = [sb_slot.tile([128, 4096], bf16, tag="slot", name=f"slot{i}")
                 for i in range(GPC)]
        y1s = [sb_y1.tile([128, 4096], bf16, tag="y1", name=f"y1_{i}")
               for i in range(GPC + 1)]
        poolb = sb.tile([128, 4096], bf16, tag="poolb")
        hmax = sb.tile([128, 4096], bf16, tag="hmax")
        aggsb = sb.tile([128, 4096], f32r, tag="aggsb")
        zt = sb.tile([128, 4096], bf16, tag="zt")
        gbt = sb.tile([128, 24], f32, tag="gbt")
        sgn = sb.tile([128, 6], f32, tag="sgn")
        disn = sb.tile([128, 16], f32, tag="disn")
        disrow = sb.tile([128, 2048], f32, tag="disrow")
        ident = sb.tile([128, 128], bf16, tag="ident")
        ebn = sb.tile([128, 192], f32, tag="ebn")    # [cot][g][chunk4] x 6
        pbn = sb.tile([128, 48], f32, tag="pbn")     # [cot][chunk4] x 6
        pack = sb.tile([128, 8], f32, tag="pack")
        glob = sb.tile([128, 8], f32, tag="glob")

        # ---- DRAM bounce tiles for collectives -------------------------
        m1_in = dram.tile([128, 4096], bf16, tag="m1i")
        m1_out = dram.tile([128, 4096], bf16, tag="m1o")
        pm_in = dram.tile([128, 2048], bf16, tag="pmi")
        pm_out = dram.tile([128, 2048], bf16, tag="pmo")
        st_in = dram.tile([128, 8], f32, tag="sti")
        st_out = dram.tile([128, 8], f32, tag="sto")

        xsh_d, ash_d, w_d, gb_d, sg_d, dn_d, dr_d, id_d, out_d = (
            io["xsh"], io["Ash"], io["Wmats"], io["gbs"], io["sgn"],
            io["disn"], io["disrow"], io["ident"], io["out"])

        sync.dma_start(gbt[:], gb_d[:, :])
        sync.dma_start(sgn[:], sg_d[:, :])
        sync.dma_start(disn[:], dn_d[:, :])
        sync.dma_start(disrow[:], dr_d[:, :])
        sync.dma_start(ident[:], id_d[:, :])

        # ---- L1 input: node-major dis-scaled x, bf16, direct from host --
        for g in range(GPC):
            sync.dma_start(slots[g][:, 0:2048], xsh_d[g, :, :])

        # A matrix: chunked load so L1 matmuls can start early
        for st in range(LT):
            sync.dma_start(Ab[:, st * L:(st + 1) * L], ash_d[:, st * L:(st + 1) * L])

        # ---- L1 pooled input: local max over copies, then pair-max ------
        vec.tensor_max(zt[:, 0:2048], slots[0][:, 0:2048], slots[1][:, 0:2048])
        vec.tensor_max(zt[:, 0:2048], zt[:, 0:2048], slots[2][:, 0:2048])
        vec.tensor_max(zt[:, 0:2048], zt[:, 0:2048], slots[3][:, 0:2048])
        sync.dma_start(pm_in[:], zt[:, 0:2048])
        _cc(nc, "AllReduce", OP.max, PAIRS, pm_in, pm_out)
        sync.dma_start(poolb[:, 0:2048], pm_out[:])

        wtiles = []
        for li in range(3):
            we = sb_w.tile([128, 512], f32r, tag="w", name=f"we{li}")
            wp = sb_w.tile([128, 512], f32r, tag="w", name=f"wp{li}")
            sync.dma_start(we[:], w_d[li, :, :])
            sync.dma_start(wp[:], w_d[3 + li, :, :])
            wtiles.append((we, wp))

        for li in range(3):
            last = (li == 2)
            ctn = 1 if li == 0 else 2
            we, wp = wtiles[li]

            # ================= phase 1: matmuls ==========================
            def agg_unit(src, ct, half):
                # agg[:, ct*2048 + half*AGGC ...]: contraction over src tiles
                pa = ps_a.tile([128, AGGC], f32, tag="psa")
                for st in range(LT):
                    lhs = src[:, st * ctn * 128 + ct * 128:
                              st * ctn * 128 + ct * 128 + 128]
                    for q in range(AGGC // 512):
                        te.matmul(
                            pa[:, q * 512:(q + 1) * 512],
                            lhs,
                            Ab[:, st * L + half * AGGC + q * 512:
                               st * L + half * AGGC + q * 512 + 512],
                            start=(st == 0), stop=(st == LT - 1))
                dsl = slice(ct * 2048 + half * AGGC,
                            ct * 2048 + half * AGGC + AGGC)
                act.activation(aggsb[:, dsl], pa[:], AF.Copy)

            def w_unit(g, wmat, cc4):
                # one 512-col chunk of y = W^T agg, both cot tiles
                for cot in range(2):
                    py = ps_y.tile([128, 512], f32, tag="psy")
                    for ct in range(ctn):
                        te.matmul(
                            py[:],
                            wmat[:, ct * 256 + cot * 128:
                                 ct * 256 + cot * 128 + 128],
                            aggsb[:, ct * 2048 + cc4 * 512:
                                  ct * 2048 + cc4 * 512 + 512],
                            start=(ct == 0), stop=(ct == ctn - 1))
                    ysl = slice(cot * 2048 + cc4 * 512,
                                cot * 2048 + cc4 * 512 + 512)
                    drw = disrow[:, cc4 * 512:(cc4 + 1) * 512]
                    if g < GPC:
                        idx = cot * 16 + g * 4 + cc4
                        vec.scalar_tensor_tensor(
                            y1s[g][:, ysl], py[:],
                            sgn[:, 2 * li + cot: 2 * li + cot + 1], drw,
                            OP.mult, OP.mult)
                        vec.bn_stats(ebn[:, idx * 6:(idx + 1) * 6], y1s[g][:, ysl])
                    else:
                        idx = cot * 4 + cc4
                        vec.tensor_tensor(y1s[GPC][:, ysl], py[:], drw, OP.mult)
                        vec.bn_stats(pbn[:, idx * 6:(idx + 1) * 6], y1s[GPC][:, ysl])

            for g in range(GPC + 1):
                src = slots[g] if g < GPC else poolb
                wmat = we if g < GPC else wp
                for half in range(2):
                    for ct in range(ctn):
                        agg_unit(src, ct, half)
                    w_unit(g, wmat, 2 * half)
                    w_unit(g, wmat, 2 * half + 1)

            # ================= phase 2: collectives ======================
            # stats first (affine blocks on it); pooled pair-max second
            # (consumed later, overlaps element P3)
            mvt = sb_small.tile([128, 2], f32, tag="sm")
            sq = sb_small.tile([128, 1], f32, tag="sm")
            for cot in range(2):
                # element branch: S1 = n*mean*s, S2 = n*(var + mean^2)
                vec.bn_aggr(mvt[:], ebn[:, cot * 96:(cot + 1) * 96])
                vec.tensor_tensor(pack[:, cot:cot + 1], mvt[:, 0:1],
                                  sgn[:, 2 * li + cot: 2 * li + cot + 1], OP.mult)
                vec.tensor_scalar(pack[:, cot:cot + 1], pack[:, cot:cot + 1],
                                  float(GPC * L), None, OP.mult)
                vec.tensor_tensor(sq[:], mvt[:, 0:1], mvt[:, 0:1], OP.mult)
                vec.tensor_tensor(sq[:], mvt[:, 1:2], sq[:], OP.add)
                vec.tensor_scalar(pack[:, 2 + cot:3 + cot], sq[:],
                                  float(GPC * L), None, OP.mult)
                # pooled branch
                vec.bn_aggr(mvt[:], pbn[:, cot * 24:(cot + 1) * 24])
                vec.tensor_scalar(pack[:, 4 + cot:5 + cot], mvt[:, 0:1],
                                  float(L), None, OP.mult)
                vec.tensor_tensor(sq[:], mvt[:, 0:1], mvt[:, 0:1], OP.mult)
                vec.tensor_tensor(sq[:], mvt[:, 1:2], sq[:], OP.add)
                vec.tensor_scalar(pack[:, 6 + cot:7 + cot], sq[:],
                                  float(L), None, OP.mult)
            sync.dma_start(st_in[:], pack[:])
            _cc(nc, "AllReduce", OP.add, ALL8, st_in, st_out)
            sync.dma_start(glob[:], st_out[:])

            if not last:
                # localmax of s*y1 over the 4 local copies -> pair max
                vec.tensor_max(zt[:], y1s[0][:], y1s[1][:])
                vec.tensor_max(zt[:], zt[:], y1s[2][:])
                vec.tensor_max(zt[:], zt[:], y1s[3][:])
                sync.dma_start(m1_in[:], zt[:])
                _cc(nc, "AllReduce", OP.max, PAIRS, m1_in, m1_out)
                sync.dma_start(hmax[:], m1_out[:])

            # affine coefficients: a = g*rsqrt(var+eps), b = be - a*mean
            eps_t = sb_small.tile([128, 1], f32, tag="sm")
            vec.memset(eps_t[:], EPS)
            me = sb_small.tile([128, 2], f32, tag="sm")
            ve = sb_small.tile([128, 2], f32, tag="sm")
            t0 = sb_small.tile([128, 2], f32, tag="sm")
            a1 = sb_small.tile([128, 2], f32, tag="sm")
            b1 = sb_small.tile([128, 2], f32, tag="sm")
            mp = sb_small.tile([128, 2], f32, tag="sm")
            vp = sb_small.tile([128, 2], f32, tag="sm")
            a2 = sb_small.tile([128, 2], f32, tag="sm")
            bs = sb_small.tile([128, 2], f32, tag="sm")

            def affine(a_t, b_t, m_t, v_t, s1_ap, s2_ap, inv_cnt, gslc, beslc):
                vec.tensor_scalar(m_t[:], s1_ap, inv_cnt, None, OP.mult)
                vec.tensor_scalar(v_t[:], s2_ap, inv_cnt, None, OP.mult)
                vec.tensor_tensor(t0[:], m_t[:], m_t[:], OP.mult)
                vec.tensor_tensor(v_t[:], v_t[:], t0[:], OP.subtract)
                act.activation(t0[:], v_t[:], AF.Sqrt, bias=eps_t[:])
                vec.reciprocal(t0[:], t0[:])
                vec.tensor_tensor(a_t[:], gslc, t0[:], OP.mult)
                vec.tensor_tensor(t0[:], a_t[:], m_t[:], OP.mult)
                vec.tensor_tensor(b_t[:], beslc, t0[:], OP.subtract)

            affine(a1, b1, me, ve, glob[:, 0:2], glob[:, 2:4], 1.0 / CNT_E,
                   gbt[:, 4 * li: 4 * li + 2], gbt[:, 4 * li + 2: 4 * li + 4])
            affine(a2, bs, mp, vp, glob[:, 4:6], glob[:, 6:8], 1.0 / CNT_P,
                   gbt[:, 12 + 4 * li: 14 + 4 * li], gbt[:, 14 + 4 * li: 16 + 4 * li])
            vec.tensor_tensor(bs[:], b1[:], bs[:], OP.add)  # b1+b2 combined

            # y2b = a2*y2 + (b1+b2), in place on pooled y (bf16)
            for cot in range(2):
                vec.tensor_scalar(y1s[GPC][:, cot * 2048:(cot + 1) * 2048],
                                  y1s[GPC][:, cot * 2048:(cot + 1) * 2048],
                                  a2[:, cot:cot + 1], bs[:, cot:cot + 1],
                                  OP.mult, OP.add)

            # ================= phase 3: x' = relu(a1*ys + y2b) ===========
            def p3_copy(src_y, dst_slot):
                for cot in range(2):
                    vec.scalar_tensor_tensor(
                        zt[:, cot * 2048:(cot + 1) * 2048],
                        src_y[:, cot * 2048:(cot + 1) * 2048],
                        a1[:, cot:cot + 1],
                        y1s[GPC][:, cot * 2048:(cot + 1) * 2048],
                        OP.mult, OP.add)
                for lt in range(LT):
                    pt = ps_t.tile([128, 256], bf16, tag="pst")
                    te.transpose(pt[:, 0:128],
                                 zt[:, lt * 128:(lt + 1) * 128], ident[:])
                    te.transpose(pt[:, 128:256],
                                 zt[:, 2048 + lt * 128: 2048 + (lt + 1) * 128],
                                 ident[:])
                    act.activation(dst_slot[:, lt * 256:(lt + 1) * 256],
                                   pt[:], AF.Relu, scale=disn[:, lt:lt + 1])

            for g in range(GPC):
                if not last:
                    p3_copy(y1s[g], slots[g])
                else:
                    # two independent per-cot chains (zt/aggsb halves),
                    # DMA out in 2 chunks each so transfers start early
                    for cot in range(2):
                        vec.scalar_tensor_tensor(
                            zt[:, cot * 2048:(cot + 1) * 2048],
                            y1s[g][:, cot * 2048:(cot + 1) * 2048],
                            a1[:, cot:cot + 1],
                            y1s[GPC][:, cot * 2048:(cot + 1) * 2048],
                            OP.mult, OP.add)
                        for hh in range(2):
                            csl = slice(cot * 2048 + hh * 1024,
                                        cot * 2048 + hh * 1024 + 1024)
                            act.activation(aggsb[:, csl], zt[:, csl], AF.Relu)
                            sync.dma_start(
                                out_d[g, cot * 128:(cot + 1) * 128,
                                      hh * 1024:(hh + 1) * 1024],
                                aggsb.bitcast(f32)[:, csl])

            if not last:
                p3_copy(hmax, poolb)


def _build():
    key = ("nc", _NO_CC)
    if key in _CACHE:
        return _CACHE[key]
    nc = bacc.Bacc("TRN2", target_bir_lowering=False, debug=False,
                   num_devices=NCORES)
    io = {
        "xsh": nc.dram_tensor("xsh", [GPC, 128, 2048], bf16, kind="ExternalInput"),
        "Ash": nc.dram_tensor("Ash", [128, LT * L], bf16, kind="ExternalInput"),
        "Wmats": nc.dram_tensor("Wmats", [6, 128, 512], f32r, kind="ExternalInput"),
        "gbs": nc.dram_tensor("gbs", [128, 24], f32, kind="ExternalInput"),
        "sgn": nc.dram_tensor("sgn", [128, 6], f32, kind="ExternalInput"),
        "disn": nc.dram_tensor("disn", [128, 16], f32, kind="ExternalInput"),
        "disrow": nc.dram_tensor("disrow", [128, 2048], f32, kind="ExternalInput"),
        "ident": nc.dram_tensor("ident", [128, 128], bf16, kind="ExternalInput"),
        "out": nc.dram_tensor("out", [GPC, 256, L], f32, kind="ExternalOutput"),
    }
    with tile.TileContext(nc) as tc:
        _emit(tc, nc, io)
    nc.compile()
    _CACHE[key] = nc
    return nc


import ml_dtypes


def _tf32(a):
    """Round f32 to TF32 (10-bit mantissa, RNE) — fp32r's precision."""
    u = np.ascontiguousarray(a, np.float32).view(np.uint32)
    r = (u + np.uint32(0xFFF) + ((u >> np.uint32(13)) & np.uint32(1))) & np.uint32(0xFFFFE000)
    return r.view(np.float32)


def _bf(a):
    """f32 array -> ml_dtypes.bfloat16 (RNE)."""
    return np.ascontiguousarray(a, np.float32).astype(ml_dtypes.bfloat16)


def _host_prep(x, edge_index, Ws, gs, bes):
    src = np.asarray(edge_index[0], dtype=np.int64)
    dst = np.asarray(edge_index[1], dtype=np.int64)
    deg = np.zeros(L, np.float32)
    np.add.at(deg, dst, np.float32(1.0))
    deg += np.float32(2.0)
    dis = (1.0 / np.sqrt(deg.astype(np.float64))).astype(np.float32)

    Acnt = np.zeros((L, L), np.float32)
    np.add.at(Acnt, (src, dst), np.float32(1.0))
    Acnt[np.arange(L), np.arange(L)] += np.float32(2.0)
    # layout [p=src%128, st*2048 + dst]
    ash = np.ascontiguousarray(
        Acnt.reshape(LT, 128, L).transpose(1, 0, 2).reshape(128, LT * L))
    ash = _bf(ash)

    wm = np.zeros((6, 128, 512), np.float32)
    for i, W in enumerate(Ws):
        cin = W.shape[0]
        wm[i, :, : (cin // 128) * 256] = np.ascontiguousarray(
            W.reshape(cin // 128, 128, 256).transpose(1, 0, 2).reshape(128, -1))
    wm = _tf32(wm)

    gb = np.zeros((128, 24), np.float32)
    sg = np.ones((128, 6), np.float32)
    # element gammas: store |g|, sign separately
    evecs = [np.abs(gs[0]), bes[0], np.abs(gs[1]), bes[1], np.abs(gs[2]), bes[2]]
    pvecs = [gs[3], bes[3], gs[4], bes[4], gs[5], bes[5]]
    for v, w in enumerate(evecs + pvecs):
        gb[:, v * 2 + 0] = w[0:128]
        gb[:, v * 2 + 1] = w[128:256]
    for li in range(3):
        s = np.where(gs[li] >= 0, np.float32(1.0), np.float32(-1.0))
        sg[:, li * 2 + 0] = s[0:128]
        sg[:, li * 2 + 1] = s[128:256]

    dn = np.ascontiguousarray(dis.reshape(LT, 128).T)          # [128, 16]
    dr = np.broadcast_to(dis[None, :], (128, L)).copy()        # [128, 2048]
    ident = _bf(np.eye(128, dtype=np.float32))
    return ash, wm, gb, sg, dn, dr, ident, dis


def kernel(x, edge_index, W1, b1, W2, b2, W3, b3, W1s, b1s, W2s, b2s, W3s, b3s,
           g1, be1, g2, be2, g3, be3, g1s, be1s, g2s, be2s, g3s, be3s):
    x = np.asarray(x, np.float32)
    ash, wm, gb, sg, dn, dr, ident, dis = _host_prep(
        x, np.asarray(edge_index),
        [np.asarray(W1, np.float32), np.asarray(W2, np.float32),
         np.asarray(W3, np.float32), np.asarray(W1s, np.float32),
         np.asarray(W2s, np.float32), np.asarray(W3s, np.float32)],
        [np.asarray(g1, np.float32), np.asarray(g2, np.float32),
         np.asarray(g3, np.float32), np.asarray(g1s, np.float32),
         np.asarray(g2s, np.float32), np.asarray(g3s, np.float32)],
        [np.asarray(be1, np.float32), np.asarray(be2, np.float32),
         np.asarray(be3, np.float32), np.asarray(be1s, np.float32),
         np.asarray(be2s, np.float32), np.asarray(be3s, np.float32)])

    # xs = dis * x, node-major bf16: [copy, p=node%128, st*128 + ch]
    xs = x * dis[None, None, None, :]
    xr = xs.reshape(B * N, CH[0], L).reshape(NCORES, GPC, CH[0], L)
    # node-major: [g, ch, st*128+p] -> [g, p, st*128 + ch]
    in_maps = []
    for k in range(NCORES):
        xm = xr[k].reshape(GPC, CH[0], LT, 128).transpose(0, 3, 2, 1)  # g,p,st,ch
        xm = np.ascontiguousarray(xm.reshape(GPC, 128, LT * CH[0]))
        in_maps.append({
            "xsh": _bf(xm),
            "Ash": ash, "Wmats": wm, "gbs": gb, "sgn": sg,
            "disn": dn, "disrow": dr, "ident": ident,
        })

    nc = _build()

    if _SIMULATE:
        from concourse.bass_interp import MultiCoreSim
        sim = MultiCoreSim(nc, NCORES)
        for k in range(NCORES):
            for nm, arr in in_maps[k].items():
                sim.cores[k].tensor(nm)[:] = arr
        sim.simulate(check_with_hw=False)
        outs = [np.array(sim.cores[k].mem_tensor("out")).reshape(GPC, 256, L)
                for k in range(NCORES)]
        return np.concatenate(outs, axis=0)

    res = run_bass_kernel_spmd(nc, in_maps, core_ids=list(range(NCORES)),
                               trace=_PROFILE)
    if _PROFILE:
        _CACHE["last_result"] = res
    outs = [np.asarray(res.results[k]["out"]).reshape(GPC, 256, L)
            for k in range(NCORES)]
    return np.concatenate(outs, axis=0).astype(np.float32)
